# Initial kernel scaffold
#
"""Trainium2 Bass kernel for nn_EntityCell (scatter_memory).

Math (per batch row b, entity e):
    gates = sigmoid(sum_d(s * (h + k)))              [B, E]
    h_tilda = tanh(h @ U + k @ V + (s @ W)[:, None]) [B, E, D]
    updated = h + gates[:, :, None] * h_tilda
    out = updated / sqrt(max(sum_d(updated^2), 1e-12))

Sharding: pure data parallel over the batch dim across 8 NeuronCores.

Per-core dataflow (B_loc=1024 rows, processed in 8 chunks of 128):
  - HWDGE DMA loads fp32 chunks; ScalarE casts to fp16.
  - DMA xbar transposes (fp16, per-entity 128x128) produce d-major tiles for PE.
  - GpSimd computes (hT+kT) and *sT (never contends with DVE ports).
  - PE: per-entity matmuls hT_e@U + kT_e@V + sT@W accumulated in PSUM
    (fp16 in, fp32 accum); gate reduction via lhsT=t2T_e, rhs=ones.
  - ScalarE: tanh on 512-wide PSUM tiles -> fp16; sigmoid for gates.
  - VectorE: fused scalar_tensor_tensor update u = g*h_tilda + h, bn_stats
    based sum(u^2), Newton rsqrt (bit-trick seed), final scale to fp32.
"""

import numpy as np

B, E, D = 8192, 20, 128
N_CORES = 8
B_LOC = B // N_CORES
CHUNK = 128
N_CHUNKS = B_LOC // CHUNK
EG = 4  # entities per psum group (4*128 fp32 = one 2KB PSUM bank)

_CACHE = {}


def _build_nc():
    import concourse.bass as bass
    import concourse.tile as tile
    from concourse import mybir
    from contextlib import ExitStack

    fp32 = mybir.dt.float32
    fp16 = mybir.dt.float16
    int32 = mybir.dt.int32
    AF = mybir.ActivationFunctionType
    OP = mybir.AluOpType

    nc = bass.Bass()
    enc_d = nc.declare_dram_parameter("enc", [B_LOC, D], fp32, isOutput=False)
    prev_d = nc.declare_dram_parameter("prev", [B_LOC, E, D], fp32, isOutput=False)
    keys_d = nc.declare_dram_parameter("keys", [B_LOC, E, D], fp32, isOutput=False)
    u_d = nc.declare_dram_parameter("U", [D, D], fp32, isOutput=False)
    v_d = nc.declare_dram_parameter("V", [D, D], fp32, isOutput=False)
    w_d = nc.declare_dram_parameter("W", [D, D], fp32, isOutput=False)
    out_d = nc.declare_dram_parameter("out", [B_LOC, E, D], fp32, isOutput=True)

    with ExitStack() as ctx:
        tc = ctx.enter_context(tile.TileContext(nc))
        const_pool = ctx.enter_context(tc.tile_pool(name="const", bufs=1))
        io_pool = ctx.enter_context(tc.tile_pool(name="io", bufs=2))
        bf_pool = ctx.enter_context(tc.tile_pool(name="bf", bufs=2))
        tr_pool = ctx.enter_context(tc.tile_pool(name="tr", bufs=2))
        sm_pool = ctx.enter_context(tc.tile_pool(name="sm", bufs=2))
        psm_pool = ctx.enter_context(tc.tile_pool(name="psm", bufs=6, space="PSUM"))
        psg_pool = ctx.enter_context(tc.tile_pool(name="psg", bufs=2, space="PSUM"))

        # ---- constants ----
        u32c = const_pool.tile([D, D], fp32)
        v32c = const_pool.tile([D, D], fp32)
        w32c = const_pool.tile([D, D], fp32)
        nc.sync.dma_start(u32c[:], u_d[:])
        nc.sync.dma_start(v32c[:], v_d[:])
        nc.sync.dma_start(w32c[:], w_d[:])
        u16c = const_pool.tile([D, D], fp16)
        v16c = const_pool.tile([D, D], fp16)
        w16c = const_pool.tile([D, D], fp16)
        nc.scalar.copy(u16c[:], u32c[:])
        nc.scalar.copy(v16c[:], v32c[:])
        nc.scalar.copy(w16c[:], w32c[:])
        ones16 = const_pool.tile([D, 1], fp16)
        nc.gpsimd.memset(ones16[:], 1.0)
        magic = const_pool.tile([CHUNK, E], int32)
        nc.gpsimd.memset(magic[:], 0x5F3759DF)

        for c in range(N_CHUNKS):
            c0 = c * CHUNK
            # ---- loads (fp32) ----
            h32 = io_pool.tile([CHUNK, E, D], fp32)
            nc.sync.dma_start(h32[:], prev_d[c0 : c0 + CHUNK])
            k32 = io_pool.tile([CHUNK, E, D], fp32)
            nc.sync.dma_start(k32[:], keys_d[c0 : c0 + CHUNK])
            s32 = io_pool.tile([CHUNK, D], fp32)
            nc.sync.dma_start(s32[:], enc_d[c0 : c0 + CHUNK])

            # ---- casts to fp16 (ScalarE) ----
            h16 = bf_pool.tile([CHUNK, E, D], fp16)
            nc.scalar.copy(h16[:], h32[:])
            k16 = bf_pool.tile([CHUNK, E, D], fp16)
            nc.scalar.copy(k16[:], k32[:])
            s16 = bf_pool.tile([CHUNK, D], fp16)
            nc.scalar.copy(s16[:], s32[:])

            # ---- DMA xbar transposes to d-major ----
            hT = tr_pool.tile([D, E, CHUNK], fp16)
            kT = tr_pool.tile([D, E, CHUNK], fp16)
            for e in range(E):
                nc.sync.dma_start(out=hT[:, e], in_=h16[:, e], transpose=True)
                nc.sync.dma_start(out=kT[:, e], in_=k16[:, e], transpose=True)
            sT = tr_pool.tile([D, CHUNK], fp16)
            nc.sync.dma_start(out=sT[:], in_=s16[:], transpose=True)

            # ---- gates input: t2T = (hT + kT) * sT  (GpSimd) ----
            hkT = tr_pool.tile([D, E, CHUNK], fp16)
            nc.gpsimd.tensor_tensor(hkT[:], hT[:], kT[:], OP.add)
            t2T = tr_pool.tile([D, E, CHUNK], fp16)
            sTb = sT[:].unsqueeze(1).broadcast_to([D, E, CHUNK])
            nc.gpsimd.tensor_tensor(t2T[:], hkT[:], sTb, OP.mult)

            # ---- gates reduce over d on PE; sigmoid on ScalarE ----
            gps = psg_pool.tile([CHUNK, E], fp32)
            for e in range(E):
                nc.tensor.matmul(
                    gps[:, e : e + 1], t2T[:, e], ones16[:], start=True, stop=True
                )
            g32 = sm_pool.tile([CHUNK, E], fp32)
            nc.scalar.activation(g32[:], gps[:], AF.Sigmoid)

            # ---- main matmuls + tanh ----
            ht16 = bf_pool.tile([CHUNK, E, D], fp16)
            for gi in range(E // EG):
                ps = psm_pool.tile([CHUNK, EG, D], fp32)
                for j in range(EG):
                    e = gi * EG + j
                    nc.tensor.matmul(ps[:, j], hT[:, e], u16c[:], start=True, stop=False)
                    nc.tensor.matmul(ps[:, j], kT[:, e], v16c[:], start=False, stop=False)
                    nc.tensor.matmul(ps[:, j], sT[:], w16c[:], start=False, stop=True)
                nc.scalar.activation(
                    ht16[:, gi * EG : (gi + 1) * EG], ps[:], AF.Tanh
                )

            # ---- update u = g * h_tilda + h (VectorE, fused) ----
            u16 = bf_pool.tile([CHUNK, E, D], fp16)
            for e in range(E):
                nc.vector.scalar_tensor_tensor(
                    u16[:, e], ht16[:, e], g32[:, e : e + 1], h16[:, e],
                    OP.mult, OP.add,
                )

            # ---- sum(u^2) via bn_stats ----
            bn = sm_pool.tile([CHUNK, E, 6], fp32)
            for gi in range(E // EG):
                nc.vector.bn_stats(
                    bn[:, gi * EG : (gi + 1) * EG, :], u16[:, gi * EG : (gi + 1) * EG]
                )
            # normsq = 64*(mu_even^2 + mu_odd^2) + (cvar_even + cvar_odd)
            t_a = sm_pool.tile([CHUNK, E], fp32)
            nc.vector.tensor_tensor(t_a[:], bn[:, :, 1], bn[:, :, 1], OP.mult)
            t_b = sm_pool.tile([CHUNK, E], fp32)
            nc.vector.tensor_tensor(t_b[:], bn[:, :, 4], bn[:, :, 4], OP.mult)
            t_ab = sm_pool.tile([CHUNK, E], fp32)
            nc.vector.tensor_tensor(t_ab[:], t_a[:], t_b[:], OP.add)
            t_c = sm_pool.tile([CHUNK, E], fp32)
            nc.vector.tensor_tensor(t_c[:], bn[:, :, 2], bn[:, :, 5], OP.add)
            a32 = sm_pool.tile([CHUNK, E], fp32)
            nc.vector.scalar_tensor_tensor(
                a32[:], t_ab[:], 64.0, t_c[:], OP.mult, OP.add
            )
            nc.vector.tensor_scalar(a32[:], a32[:], 1e-12, None, op0=OP.max)

            # ---- r = rsqrt(a) : bit-trick seed + 2 Newton iterations ----
            ti = sm_pool.tile([CHUNK, E], int32)
            nc.vector.tensor_scalar(
                ti[:], a32[:].bitcast(int32), 1, None, op0=OP.logical_shift_right
            )
            yi = sm_pool.tile([CHUNK, E], int32)
            nc.vector.tensor_tensor(yi[:], magic[:], ti[:], OP.subtract)
            y = yi[:].bitcast(fp32)
            for _ in range(2):
                y2 = sm_pool.tile([CHUNK, E], fp32)
                nc.vector.tensor_tensor(y2[:], y, y, OP.mult)
                tt = sm_pool.tile([CHUNK, E], fp32)
                nc.vector.tensor_tensor(tt[:], a32[:], y2[:], OP.mult)
                ww = sm_pool.tile([CHUNK, E], fp32)
                nc.vector.tensor_scalar(ww[:], tt[:], -0.5, 1.5, op0=OP.mult, op1=OP.add)
                yn = sm_pool.tile([CHUNK, E], fp32)
                nc.vector.tensor_tensor(yn[:], y, ww[:], OP.mult)
                y = yn[:]

            # ---- scale out = u * r (fp32 out) and store ----
            o32 = io_pool.tile([CHUNK, E, D], fp32)
            for e in range(E):
                nc.vector.scalar_tensor_tensor(
                    o32[:, e], u16[:, e], y[:, e : e + 1], u16[:, e],
                    OP.mult, OP.bypass,
                )
            nc.sync.dma_start(out=out_d[c0 : c0 + CHUNK], in_=o32[:])

    return nc


def _get_nc():
    if "nc" not in _CACHE:
        _CACHE["nc"] = _build_nc()
    return _CACHE["nc"]


def kernel(encoded_sents, prev_states, keys, U, V, W):
    from concourse.bass_utils import run_bass_kernel_spmd

    nc = _get_nc()
    enc = np.ascontiguousarray(np.asarray(encoded_sents, dtype=np.float32))
    prev = np.ascontiguousarray(np.asarray(prev_states, dtype=np.float32))
    kys = np.ascontiguousarray(np.asarray(keys, dtype=np.float32))
    U = np.ascontiguousarray(np.asarray(U, dtype=np.float32))
    V = np.ascontiguousarray(np.asarray(V, dtype=np.float32))
    W = np.ascontiguousarray(np.asarray(W, dtype=np.float32))

    in_maps = []
    for i in range(N_CORES):
        lo, hi = i * B_LOC, (i + 1) * B_LOC
        in_maps.append(
            {
                "enc": enc[lo:hi],
                "prev": prev[lo:hi],
                "keys": kys[lo:hi],
                "U": U,
                "V": V,
                "W": W,
            }
        )

    res = run_bass_kernel_spmd(nc, in_maps, list(range(N_CORES)))
    out = np.concatenate([res.results[i]["out"] for i in range(N_CORES)], axis=0)
    return out.astype(np.float32)


# revision 12
# speedup vs baseline: 5.5311x; 5.5311x over previous
"""Trainium2 Bass kernel for nn_EntityCell (scatter_memory).

Math (per batch row b, entity e):
    gates = sigmoid(sum_d(s * (h + k)))              [B, E]
    h_tilda = tanh(h @ U + k @ V + (s @ W)[:, None]) [B, E, D]
    updated = h + gates[:, :, None] * h_tilda
    out = updated / sqrt(max(sum_d(updated^2), 1e-12))

Sharding: pure data parallel over the batch dim across 8 NeuronCores.

Per-core dataflow (B_loc=1024 rows, processed in 8 chunks of 128):
  - HWDGE DMA loads fp32 chunks; ScalarE casts to fp16.
  - DMA xbar transposes (fp16, per-entity 128x128) produce d-major tiles for PE.
  - GpSimd computes (hT+kT) and *sT (never contends with DVE ports).
  - PE: per-entity matmuls hT_e@U + kT_e@V + sT@W accumulated in PSUM
    (fp16 in, fp32 accum); gate reduction via lhsT=t2T_e, rhs=ones.
  - ScalarE: tanh on 512-wide PSUM tiles -> fp16; sigmoid for gates.
  - VectorE: fused scalar_tensor_tensor update u = g*h_tilda + h, bn_stats
    based sum(u^2), Newton rsqrt (bit-trick seed), final scale to fp32.
"""

import numpy as np

B, E, D = 8192, 20, 128
N_CORES = 8
B_LOC = B // N_CORES
CHUNK = 128
N_CHUNKS = B_LOC // CHUNK
EG = 4  # entities per psum group (4*128 fp32 = one 2KB PSUM bank)

_CACHE = {}


def _build_nc(reps=1):
    import concourse.bass as bass
    import concourse.tile as tile
    from concourse import bacc, mybir
    from contextlib import ExitStack

    fp32 = mybir.dt.float32
    fp16 = mybir.dt.float16
    int32 = mybir.dt.int32
    AF = mybir.ActivationFunctionType
    OP = mybir.AluOpType

    nc = bacc.Bacc("TRN2", target_bir_lowering=False, debug=False)
    enc_d = nc.declare_dram_parameter("enc", [B_LOC, D], fp32, isOutput=False)
    prev_d = nc.declare_dram_parameter("prev", [B_LOC, E, D], fp32, isOutput=False)
    keys_d = nc.declare_dram_parameter("keys", [B_LOC, E, D], fp32, isOutput=False)
    u_d = nc.declare_dram_parameter("U", [D, D], fp32, isOutput=False)
    v_d = nc.declare_dram_parameter("V", [D, D], fp32, isOutput=False)
    w_d = nc.declare_dram_parameter("W", [D, D], fp32, isOutput=False)
    out_d = nc.declare_dram_parameter("out", [B_LOC, E, D], fp32, isOutput=True)

    with ExitStack() as ctx:
        tc = ctx.enter_context(tile.TileContext(nc))
        const_pool = ctx.enter_context(tc.tile_pool(name="const", bufs=1))
        io_pool = ctx.enter_context(tc.tile_pool(name="io", bufs=2))
        bf_pool = ctx.enter_context(tc.tile_pool(name="bf", bufs=2))
        tr_pool = ctx.enter_context(tc.tile_pool(name="tr", bufs=2))
        sm_pool = ctx.enter_context(tc.tile_pool(name="sm", bufs=2))
        psm_pool = ctx.enter_context(tc.tile_pool(name="psm", bufs=6, space="PSUM"))
        psg_pool = ctx.enter_context(tc.tile_pool(name="psg", bufs=2, space="PSUM"))

        # ---- constants ----
        u32c = const_pool.tile([D, D], fp32)
        v32c = const_pool.tile([D, D], fp32)
        w32c = const_pool.tile([D, D], fp32)
        nc.sync.dma_start(u32c[:], u_d[:])
        nc.sync.dma_start(v32c[:], v_d[:])
        nc.sync.dma_start(w32c[:], w_d[:])
        u16c = const_pool.tile([D, D], fp16)
        v16c = const_pool.tile([D, D], fp16)
        w16c = const_pool.tile([D, D], fp16)
        nc.scalar.copy(u16c[:], u32c[:])
        nc.scalar.copy(v16c[:], v32c[:])
        nc.scalar.copy(w16c[:], w32c[:])
        ones16 = const_pool.tile([D, 1], fp16)
        nc.gpsimd.memset(ones16[:], 1.0)
        magic = const_pool.tile([CHUNK, E], int32)
        nc.gpsimd.memset(magic[:], 0x5F3759DF)

        for c in range(N_CHUNKS * reps):
            c0 = (c % N_CHUNKS) * CHUNK
            # ---- loads (fp32) ----
            h32 = io_pool.tile([CHUNK, E, D], fp32)
            nc.sync.dma_start(h32[:], prev_d[c0 : c0 + CHUNK])
            k32 = io_pool.tile([CHUNK, E, D], fp32)
            nc.sync.dma_start(k32[:], keys_d[c0 : c0 + CHUNK])
            s32 = io_pool.tile([CHUNK, D], fp32)
            nc.sync.dma_start(s32[:], enc_d[c0 : c0 + CHUNK])

            # ---- casts to fp16 (ScalarE) ----
            h16 = bf_pool.tile([CHUNK, E, D], fp16)
            nc.scalar.copy(h16[:], h32[:])
            k16 = bf_pool.tile([CHUNK, E, D], fp16)
            nc.scalar.copy(k16[:], k32[:])
            s16 = bf_pool.tile([CHUNK, D], fp16)
            nc.scalar.copy(s16[:], s32[:])

            # ---- DMA xbar transposes to d-major ----
            hT = tr_pool.tile([D, E, CHUNK], fp16)
            kT = tr_pool.tile([D, E, CHUNK], fp16)
            for gi in range(E // EG):
                sl = slice(gi * EG, (gi + 1) * EG)
                nc.sync.dma_start_transpose(out=hT[:, sl, :], in_=h16[:, sl])
                nc.sync.dma_start_transpose(out=kT[:, sl, :], in_=k16[:, sl])
            sT = tr_pool.tile([D, CHUNK], fp16)
            nc.sync.dma_start(out=sT[:], in_=s16[:], transpose=True)

            # ---- gates input: t2T = (hT + kT) * sT  (GpSimd) ----
            hkT = tr_pool.tile([D, E, CHUNK], fp16)
            nc.gpsimd.tensor_tensor(hkT[:], hT[:], kT[:], OP.add)
            t2T = tr_pool.tile([D, E, CHUNK], fp16)
            sTb = sT[:].unsqueeze(1).broadcast_to([D, E, CHUNK])
            nc.gpsimd.tensor_tensor(t2T[:], hkT[:], sTb, OP.mult)

            # ---- gates reduce over d on PE; sigmoid on ScalarE ----
            gps = psg_pool.tile([CHUNK, E], fp32)
            for e in range(E):
                nc.tensor.matmul(
                    gps[:, e : e + 1], t2T[:, e], ones16[:], start=True, stop=True
                )
            g32 = sm_pool.tile([CHUNK, E], fp32)
            nc.scalar.activation(g32[:], gps[:], AF.Sigmoid)

            # ---- main matmuls + tanh ----
            ht16 = bf_pool.tile([CHUNK, E, D], fp16)
            for gi in range(E // EG):
                ps = psm_pool.tile([CHUNK, EG, D], fp32)
                for j in range(EG):
                    e = gi * EG + j
                    nc.tensor.matmul(ps[:, j], hT[:, e], u16c[:], start=True, stop=False)
                    nc.tensor.matmul(ps[:, j], kT[:, e], v16c[:], start=False, stop=False)
                    nc.tensor.matmul(ps[:, j], sT[:], w16c[:], start=False, stop=True)
                nc.scalar.activation(
                    ht16[:, gi * EG : (gi + 1) * EG], ps[:], AF.Tanh
                )

            # ---- update u = g * h_tilda + h (VectorE, fused) ----
            u16 = bf_pool.tile([CHUNK, E, D], fp16)
            for e in range(E):
                nc.vector.scalar_tensor_tensor(
                    u16[:, e], ht16[:, e], g32[:, e : e + 1], h16[:, e],
                    OP.mult, OP.add,
                )

            # ---- sum(u^2) via bn_stats ----
            bn = sm_pool.tile([CHUNK, E, 6], fp32)
            for e in range(E):
                # HW BNStats emits exactly 6 elements/partition per call
                nc.vector.bn_stats(bn[:, e, :], u16[:, e])
            # normsq = 64*(mu_even^2 + mu_odd^2) + (cvar_even + cvar_odd)
            t_a = sm_pool.tile([CHUNK, E], fp32)
            nc.vector.tensor_tensor(t_a[:], bn[:, :, 1], bn[:, :, 1], OP.mult)
            t_b = sm_pool.tile([CHUNK, E], fp32)
            nc.vector.tensor_tensor(t_b[:], bn[:, :, 4], bn[:, :, 4], OP.mult)
            t_ab = sm_pool.tile([CHUNK, E], fp32)
            nc.vector.tensor_tensor(t_ab[:], t_a[:], t_b[:], OP.add)
            t_c = sm_pool.tile([CHUNK, E], fp32)
            nc.vector.tensor_tensor(t_c[:], bn[:, :, 2], bn[:, :, 5], OP.add)
            a32 = sm_pool.tile([CHUNK, E], fp32)
            nc.vector.scalar_tensor_tensor(
                a32[:], t_ab[:], 64.0, t_c[:], OP.mult, OP.add
            )
            nc.vector.tensor_scalar(a32[:], a32[:], 1e-12, None, op0=OP.max)

            # ---- r = rsqrt(a) : bit-trick seed + 2 Newton iterations ----
            ti = sm_pool.tile([CHUNK, E], int32)
            nc.vector.tensor_scalar(
                ti[:], a32[:].bitcast(int32), 1, None, op0=OP.logical_shift_right
            )
            yi = sm_pool.tile([CHUNK, E], int32)
            nc.vector.tensor_tensor(yi[:], magic[:], ti[:], OP.subtract)
            y = yi[:].bitcast(fp32)
            for _ in range(2):
                y2 = sm_pool.tile([CHUNK, E], fp32)
                nc.vector.tensor_tensor(y2[:], y, y, OP.mult)
                tt = sm_pool.tile([CHUNK, E], fp32)
                nc.vector.tensor_tensor(tt[:], a32[:], y2[:], OP.mult)
                ww = sm_pool.tile([CHUNK, E], fp32)
                nc.vector.tensor_scalar(ww[:], tt[:], -0.5, 1.5, op0=OP.mult, op1=OP.add)
                yn = sm_pool.tile([CHUNK, E], fp32)
                nc.vector.tensor_tensor(yn[:], y, ww[:], OP.mult)
                y = yn[:]

            # ---- scale out = u * r (fp32 out) and store ----
            o32 = io_pool.tile([CHUNK, E, D], fp32)
            for e in range(E):
                nc.vector.scalar_tensor_tensor(
                    o32[:, e], u16[:, e], y[:, e : e + 1], u16[:, e],
                    OP.mult, OP.bypass,
                )
            nc.sync.dma_start(out=out_d[c0 : c0 + CHUNK], in_=o32[:])

    nc.compile()
    return nc


def _fix_xpose_waits(nc):
    """DMA instruction structs have limited sync-wait slots (1 for the
    DIRECT2D_XPOSE, 2 for the pseudo DIRECT2D), but Tile may attach more.
    Move the excess onto an SP NoOp inserted right before each DMA: SP
    executes its stream in order, so the moved waits are satisfied before
    the DMA descriptor is dispatched."""
    import bass_rust
    from concourse import mybir

    limits = {"InstDmaTransposeAnt": 1, "InstDMACopy": 1}
    cnt = 0
    for fn in nc.m.functions:
        for bb in fn.blocks:
            insts = bb.instructions
            new = []
            for inst in insts:
                si = getattr(inst, "sync_info", None)
                lim = limits.get(type(inst).__name__)
                if lim is not None and si is not None and len(si.on_wait) > lim:
                    waits = list(si.on_wait)
                    nop = bass_rust.InstNoOp(
                        name=f"xpw-{inst.name}-{cnt}", engine=inst.engine
                    )
                    nop.sync_info = mybir.SyncInfo(on_wait=waits[lim:], on_update=[])
                    inst.sync_info = mybir.SyncInfo(
                        on_wait=waits[:lim], on_update=list(si.on_update)
                    )
                    nc.register_instruction(nop, overwrite=True)
                    new.append(nop)
                    cnt += 1
                new.append(inst)
            if cnt:
                insts[:] = new
    return cnt


def _get_nc():
    if "nc" not in _CACHE:
        _CACHE["nc"] = _build_nc()
    return _CACHE["nc"]


def kernel(encoded_sents, prev_states, keys, U, V, W):
    from concourse.bass_utils import run_bass_kernel_spmd

    nc = _get_nc()
    enc = np.ascontiguousarray(np.asarray(encoded_sents, dtype=np.float32))
    prev = np.ascontiguousarray(np.asarray(prev_states, dtype=np.float32))
    kys = np.ascontiguousarray(np.asarray(keys, dtype=np.float32))
    U = np.ascontiguousarray(np.asarray(U, dtype=np.float32))
    V = np.ascontiguousarray(np.asarray(V, dtype=np.float32))
    W = np.ascontiguousarray(np.asarray(W, dtype=np.float32))

    in_maps = []
    for i in range(N_CORES):
        lo, hi = i * B_LOC, (i + 1) * B_LOC
        in_maps.append(
            {
                "enc": enc[lo:hi],
                "prev": prev[lo:hi],
                "keys": kys[lo:hi],
                "U": U,
                "V": V,
                "W": W,
            }
        )

    res = run_bass_kernel_spmd(nc, in_maps, list(range(N_CORES)))
    out = np.concatenate([res.results[i]["out"] for i in range(N_CORES)], axis=0)
    return out.astype(np.float32)
